# revision 5
# baseline (speedup 1.0000x reference)
"""Trainium2 Bass kernel for nn_EnhancementGenerator.

Math: the reference is a (buggy, non-recurrent) bidirectional 2-layer GRU
applied pointwise over (B,T), followed by an efficient-kan KANLinear and
1.2*sigmoid(slope*out).  Everything is row-pointwise except that the
backward direction pairs output row (b,t) with input row (b,T-1-t).

Reformulation:
  * GRU: no recurrence => 4 independent "cells".  Both directions are packed
    into [f(40); b(40)] = partitions 0:40 / 64:104 (partition-base rule);
    the time reversal is applied once at feat-assembly with a reversed
    free-dim access pattern.  h1 is carried negated (h1n = (z1-1)*n1); the
    L1 recurrent weights are negated on host.  x feature 256 (257 = 2*128+1)
    rides as partition 104 of the L1 h-chunk (weight row = Wih_l1[:,256]);
    for L0 and the L1 n-gate it is a K=1 matmul pass.
  * KAN: base-silu + cubic-spline basis == span{silu, 1..f^3, 4 trunc
    cubes}.  Approximated by a degree-7 polynomial in feat (silu fit is
    ~exact; trunc-cube fits give ~8.5e-3 rel overall, budget is 2e-2).
    spl = sum_d P_d f^d: 7 rhs vectors [f..f^7] x 80 feats = 560 K rows,
    DMA-repacked dense into 5 chunks [120,120,120,120,80] so the KAN is
    5 K-chunks x 3 M-blocks of matmul.  P_d / bias folded on host.
  * The final 1.2x scale is applied on host during the f16->f32 upcast.

Layout: features/gates in SBUF partitions, rows in the free dim.  Each core
gets 8 batch samples = 8000 rows; matmuls run at N=500 (PSUM f32 bank limit)
as 2 halves per sample; elementwise ops run full-width N=1000 in fp16.
"""
import os
import sys

for _p in (
    "/root/.axon_site",
    "/root/.axon_site/_ro/trn_rl_repo",
    "/root/.axon_site/_ro/pypackages",
    "/opt/trn_rl_repo",
    "/opt/pypackages",
):
    if os.path.isdir(_p) and _p not in sys.path:
        sys.path.append(_p)

import numpy as np

import concourse.bass as bass
import concourse.tile as tile
from concourse import bacc, mybir
from concourse.bass_utils import run_bass_kernel_spmd

F32 = mybir.dt.float32
FP16 = mybir.dt.float16
AF = mybir.ActivationFunctionType
ALU = mybir.AluOpType

N_CORES = 8
B, T, IN_SIZE, HID, OUT_SIZE = 64, 1000, 257, 40, 257
NT = 500            # psum half width (f32 bank limit 512)
SPB = B // N_CORES  # samples per core
PG = 104            # packed direction block: f at 0:40, b at 64:104
BO = 64             # b-direction partition offset
NPOW = 7            # polynomial degree (rhs = f..f^7)
KCHUNK = [120, 120, 120, 120, 80]   # dense KAN K-chunks (sum = 7*80)
MCH = [(0, 128), (128, 128), (256, 1)]  # KAN output M-blocks

# Degree-7 LSQ fits over the empirical feat distribution (|f|<1):
# silu(f) and the 4 two-sided truncated cubes of the uniform-knot spline.
C_SILU = [6.308492028423155e-07, 0.5000000490243806, 0.24997364172659992,
          -5.742981447318585e-07, -0.02066946270890013, 1.5669095660303758e-06,
          0.0017603356959882746, -1.1728766357584825e-06]
C_TRUNC = {
    (-0.6, "L"): [0.0001651082331822392, 0.000167641210570048,
                  -0.007694731928173372, 0.00250875223865735,
                  0.04856470985596804, -0.032699784691622734,
                  -0.07363537344626084, 0.0637407065220845],
    (-0.2, "L"): [-0.0003715091342959778, -0.005336319702092779,
                  0.03194056417245047, 0.05081939992485063,
                  -0.3584596956621296, 0.36429432612922186,
                  0.07159215401107981, -0.15726428315914862],
    (0.2, "R"): [0.00036529457231786937, -0.005363521000470467,
                 -0.03183079083350338, 0.05104781465712958,
                 0.3580816694009009, 0.3637871708587963,
                 -0.07126037134005792, -0.15693749830364792],
    (0.6, "R"): [-0.00016336387734099217, 0.00017070486860762662,
                 0.007648889140930259, 0.0024753300438013297,
                 -0.048404573931655016, -0.03263337742091405,
                 0.07349986964835029, 0.0637109540665507],
}


# --------------------------------------------------------------------------
# host-side weight folding
# --------------------------------------------------------------------------
def fold_weights(inp):
    from math import comb
    W = {k: np.asarray(v, dtype=np.float64) for k, v in inp.items()}
    out = {}

    # ---- GRU input weights: 6 gate blocks of 104 cols: (l,g) l-major
    wgi = np.zeros((IN_SIZE, 6 * PG))
    for l in range(2):
        for g in range(3):
            c0 = (l * 3 + g) * PG
            wgi[:, c0:c0 + 40] = W["Wih_f"][l][g * 40:(g + 1) * 40].T
            wgi[:, c0 + BO:c0 + BO + 40] = W["Wih_b"][l][g * 40:(g + 1) * 40].T
    out["wgi0"] = wgi[0:128]
    out["wgi1"] = wgi[128:256]
    out["w256"] = wgi[256:257]

    # ---- L1 recurrent weights (negated, blockdiag) + x256 row:
    # blocks r2 | z2 | p3(n)
    wgh = np.zeros((105, 3 * PG))
    for gi_, g in enumerate([0, 1, 2]):
        c0 = gi_ * PG
        wgh[0:40, c0:c0 + 40] = -W["Whh_f"][1][g * 40:(g + 1) * 40].T
        wgh[BO:BO + 40, c0 + BO:c0 + BO + 40] = -W["Whh_b"][1][g * 40:(g + 1) * 40].T
    # x256 contribution to L1 r,z gates rides the h-chunk (row 104)
    for gi_, g in enumerate([0, 1]):
        c0 = gi_ * PG
        wgh[104, c0:c0 + 40] = W["Wih_f"][1][g * 40:(g + 1) * 40, 256]
        wgh[104, c0 + BO:c0 + BO + 40] = W["Wih_b"][1][g * 40:(g + 1) * 40, 256]
    out["wgh"] = wgh

    # ---- GRU biases [105, 8]: cols l*4 + (r, z, bhh_n, bih_n)
    bg = np.zeros((105, 8))
    for l in range(2):
        for gi_ in range(2):
            bg[0:40, l * 4 + gi_] = (W["bih_f"][l][gi_ * 40:(gi_ + 1) * 40]
                                     + W["bhh_f"][l][gi_ * 40:(gi_ + 1) * 40])
            bg[BO:BO + 40, l * 4 + gi_] = (W["bih_b"][l][gi_ * 40:(gi_ + 1) * 40]
                                           + W["bhh_b"][l][gi_ * 40:(gi_ + 1) * 40])
        bg[0:40, l * 4 + 2] = W["bhh_f"][l][80:120]
        bg[BO:BO + 40, l * 4 + 2] = W["bhh_b"][l][80:120]
        bg[0:40, l * 4 + 3] = W["bih_f"][l][80:120]
        bg[BO:BO + 40, l * 4 + 3] = W["bih_b"][l][80:120]
    out["bgru"] = bg

    # ---- KAN: exact truncated-power decomposition, then poly-7 coefficients
    h = 0.4
    t = -2.2 + h * np.arange(12)
    w = W["spline_weight"] * W["spline_scaler"][..., None]          # (257,80,8)
    s = np.zeros((8, 12))
    for m in range(8):
        for k in range(5):
            s[m, m + k] = ((-1) ** k) * comb(4, k) / (6 * h ** 3)
    V = np.einsum("oim,mj->oij", w, s)                              # (257,80,12)
    A = np.zeros((4, 257, 80))
    for j in range(6):
        for d in range(4):
            A[d] += V[:, :, j] * comb(3, d) * ((-t[j]) ** (3 - d))
    tr_mats = {(-0.6, "L"): -V[:, :, 4], (-0.2, "L"): -V[:, :, 5],
               (0.2, "R"): V[:, :, 6], (0.6, "R"): V[:, :, 7]}
    P = np.zeros((NPOW + 1, 257, 80))
    for d in range(4):
        P[d] += A[d]
    bw = W["base_weight"]
    for d in range(NPOW + 1):
        P[d] += bw * C_SILU[d]
        for key, M in tr_mats.items():
            P[d] += M * C_TRUNC[key][d]
    slope = W["slope"]

    # dense chunk weights: flat k = (d-1)*80 + j (j = feat index 0:80)
    Pm = P * slope[None, :, None]                                   # fold slope
    flat = np.concatenate([Pm[d].T for d in range(1, NPOW + 1)], axis=0)  # (560,257)
    o0 = 0
    for c, rows in enumerate(KCHUNK):
        out[f"wk{c}"] = flat[o0:o0 + rows]
        o0 += rows

    bk = np.zeros((128, 3))
    a0 = Pm[0].sum(axis=1)                                          # (257,)
    bk[0:128, 0] = a0[0:128]
    bk[0:128, 1] = a0[128:256]
    bk[0:1, 2] = a0[256:257]
    out["bkan"] = bk

    res = {}
    for k, v in out.items():
        dt = np.float32 if k in ("bgru", "bkan") else np.float16
        res[k] = np.ascontiguousarray(v, dtype=dt)
    return res


# --------------------------------------------------------------------------
# device kernel
# --------------------------------------------------------------------------
def build_nc(n_samples=SPB):
    rows = n_samples * T
    NT2 = 2 * NT
    nc = bacc.Bacc("TRN2", target_bir_lowering=False, debug=False)

    def mm(out, lhsT, rhs, **kw):
        nc.tensor.matmul(out, lhsT, rhs, **kw)

    xt0_d = nc.dram_tensor("xt0", [128, rows], FP16, kind="ExternalInput")
    xt1_d = nc.dram_tensor("xt1", [128, rows], FP16, kind="ExternalInput")
    x2_d = nc.dram_tensor("x2", [1, rows], FP16, kind="ExternalInput")
    wgi0_d = nc.dram_tensor("wgi0", [128, 6 * PG], FP16, kind="ExternalInput")
    wgi1_d = nc.dram_tensor("wgi1", [128, 6 * PG], FP16, kind="ExternalInput")
    w256_d = nc.dram_tensor("w256", [1, 6 * PG], FP16, kind="ExternalInput")
    wgh_d = nc.dram_tensor("wgh", [105, 3 * PG], FP16, kind="ExternalInput")
    wk_d = [nc.dram_tensor(f"wk{c}", [KCHUNK[c], 257], FP16, kind="ExternalInput")
            for c in range(5)]
    bgru_d = nc.dram_tensor("bgru", [105, 8], F32, kind="ExternalInput")
    bkan_d = nc.dram_tensor("bkan", [128, 3], F32, kind="ExternalInput")
    yt_d = nc.dram_tensor("yt", [OUT_SIZE, rows], FP16, kind="ExternalOutput")

    with tile.TileContext(nc) as tc:
        with (
            tc.tile_pool(name="wts", bufs=1) as wp,
            tc.tile_pool(name="xin", bufs=3) as xp,
            tc.tile_pool(name="work", bufs=1) as kp,
            tc.tile_pool(name="outp", bufs=2) as op_,
            tc.tile_pool(name="psg", bufs=1, space="PSUM") as psg,
        ):
            # ---- resident weights
            wgi0 = wp.tile([128, 6 * PG], FP16, tag="wgi0")
            nc.sync.dma_start(wgi0[:], wgi0_d[:])
            wgi1 = wp.tile([128, 6 * PG], FP16, tag="wgi1")
            nc.sync.dma_start(wgi1[:], wgi1_d[:])
            w256 = wp.tile([1, 6 * PG], FP16, tag="w256")
            nc.sync.dma_start(w256[:], w256_d[:])
            wgh = wp.tile([105, 3 * PG], FP16, tag="wgh")
            nc.sync.dma_start(wgh[:], wgh_d[:])
            wk = []
            for c in range(5):
                wt = wp.tile([KCHUNK[c], 257], FP16, tag=f"wk{c}")
                nc.sync.dma_start(wt[:], wk_d[c][:])
                wk.append(wt)
            bg = wp.tile([105, 8], F32, tag="bgru")
            nc.sync.dma_start(bg[:], bgru_d[:])
            bk = wp.tile([128, 3], F32, tag="bkan")
            nc.sync.dma_start(bk[:], bkan_d[:])

            # HAM warmup: ~4.3us of back-to-back small matmuls flips the PE
            # clock gate to 8/8 (2.4 GHz) before the real work starts.
            warm = psg.tile([128, NT], F32, tag="pp", bufs=2, name="warm")
            for _ in range(48):
                mm(warm[:, 0:128], wgi0[:, 0:128], wgi0[:, 0:128],
                   start=True, stop=True)

            S = [dict() for _ in range(n_samples)]

            def load_x(smp):
                st = S[smp]
                s0 = smp * T
                st["x0"] = xp.tile([128, NT2], FP16, tag="x0", name="x0")
                nc.sync.dma_start(st["x0"][:], xt0_d[:, s0:s0 + NT2])
                st["x1"] = xp.tile([128, NT2], FP16, tag="x1", name="x1")
                nc.sync.dma_start(st["x1"][:], xt1_d[:, s0:s0 + NT2])
                st["x2"] = xp.tile([1, NT2], FP16, tag="x2", name="x2")
                nc.sync.dma_start(st["x2"][:], x2_d[:, s0:s0 + NT2])
                # h-chunk rhs for L1: rows 0:104 = h1n (written in L0),
                # row 104 = x feature 256
                st["htl"] = xp.tile([105, NT2], FP16, tag="htl", name="htl")
                nc.sync.dma_start(st["htl"][104:105, :], x2_d[:, s0:s0 + NT2])

            def gi3(p, st, blk, hs, last_extra=False):
                c0 = blk * PG
                mm(p[:], wgi0[:, c0:c0 + PG], st["x0"][:, hs], start=True, stop=False)
                mm(p[:], wgi1[:, c0:c0 + PG], st["x1"][:, hs], start=False,
                   stop=False)
                if not last_extra:
                    mm(p[:], w256[:, c0:c0 + PG], st["x2"][:, hs], start=False,
                       stop=True)

            def emit_l0(smp):
                st = S[smp]
                rt = kp.tile([PG, NT2], FP16, tag="rt", bufs=2)
                zt = kp.tile([PG, NT2], FP16, tag="zt", bufs=2)
                ut = kp.tile([PG, NT2], FP16, tag="ut", bufs=2)
                for h in range(2):
                    hs = slice(h * NT, (h + 1) * NT)
                    ps_r = psg.tile([PG, NT], F32, tag="l0r", name="ps_r")
                    gi3(ps_r, st, 0, hs)
                    ps_z = psg.tile([PG, NT], F32, tag="l0z", name="ps_z")
                    gi3(ps_z, st, 1, hs)
                    ps_n = psg.tile([PG, NT], F32, tag="l0n", name="ps_n")
                    gi3(ps_n, st, 2, hs)
                    nc.scalar.activation(rt[:, hs], ps_r[:], AF.Sigmoid,
                                         bias=bg[0:PG, 0:1])
                    nc.scalar.activation(zt[:, hs], ps_z[:], AF.Sigmoid,
                                         bias=bg[0:PG, 1:2])
                    nc.vector.scalar_tensor_tensor(
                        ut[:, hs], rt[:, hs], bg[0:PG, 2:3], ps_n[:],
                        op0=ALU.mult, op1=ALU.add)
                n1 = kp.tile([PG, NT2], FP16, tag="n1", bufs=2)
                nc.scalar.activation(n1[:], ut[:], AF.Tanh, bias=bg[0:PG, 3:4])
                # h1n = (z-1)*n1 = -h1, written into the L1 h-chunk rhs
                nc.vector.scalar_tensor_tensor(
                    st["htl"][0:PG, :], zt[:], 1.0, n1[:],
                    op0=ALU.subtract, op1=ALU.mult)

            def emit_l1(smp):
                st = S[smp]
                r2t = kp.tile([PG, NT2], FP16, tag="r2t", bufs=2)
                z2t = kp.tile([PG, NT2], FP16, tag="z2t", bufs=2)
                t2t = kp.tile([PG, NT2], FP16, tag="t2t", bufs=2)
                vt = kp.tile([PG, NT2], FP16, tag="vt", bufs=2)
                for h in range(2):
                    hs = slice(h * NT, (h + 1) * NT)
                    ps_r2 = psg.tile([PG, NT], F32, tag="l1r", name="ps_r2")
                    gi3(ps_r2, st, 3, hs, last_extra=True)
                    mm(ps_r2[:], wgh[:, 0:PG], st["htl"][:, hs], start=False,
                       stop=True)
                    ps_z2 = psg.tile([PG, NT], F32, tag="l1z", name="ps_z2")
                    gi3(ps_z2, st, 4, hs, last_extra=True)
                    mm(ps_z2[:], wgh[:, PG:2 * PG], st["htl"][:, hs], start=False,
                       stop=True)
                    ps_n2 = psg.tile([PG, NT], F32, tag="l1n", name="ps_n2")
                    gi3(ps_n2, st, 5, hs)
                    ps_p3 = psg.tile([128, NT], F32, tag="pp", bufs=2, name="ps_p3")
                    mm(ps_p3[0:PG, :], wgh[0:104, 2 * PG:3 * PG],
                       st["htl"][0:104, hs], start=True, stop=True)
                    nc.scalar.activation(r2t[:, hs], ps_r2[:], AF.Sigmoid,
                                         bias=bg[0:PG, 4:5])
                    nc.scalar.activation(z2t[:, hs], ps_z2[:], AF.Sigmoid,
                                         bias=bg[0:PG, 5:6])
                    nc.vector.scalar_tensor_tensor(
                        t2t[:, hs], ps_p3[0:PG, :], bg[0:PG, 6:7], r2t[:, hs],
                        op0=ALU.add, op1=ALU.mult)
                    nc.vector.tensor_add(vt[:, hs], t2t[:, hs], ps_n2[:])
                n2 = kp.tile([PG, NT2], FP16, tag="n2", bufs=2)
                nc.scalar.activation(n2[:], vt[:], AF.Tanh, bias=bg[0:PG, 7:8])
                # hf = (1-z2)*n2 + z2*h1 = -(A + Bv), A=(z2-1)*n2, Bv=z2*h1n
                A = kp.tile([PG, NT2], FP16, tag="A", bufs=2)
                nc.vector.scalar_tensor_tensor(
                    A[:], z2t[:], 1.0, n2[:], op0=ALU.subtract, op1=ALU.mult)
                Bv = kp.tile([PG, NT2], FP16, tag="Bv", bufs=2)
                nc.vector.tensor_mul(Bv[:], z2t[:], st["htl"][0:PG, :])
                # feat goes dense [80] into KAN chunk 0: f-part written directly,
                # b-part (time-reversed) via a legal base-64 scratch + 1 DMA.
                c0 = kp.tile([120, NT2], FP16, tag="dc0", bufs=3)
                fb = kp.tile([PG, NT2], FP16, tag="fb", bufs=2)
                nc.vector.scalar_tensor_tensor(
                    c0[0:40, :], A[0:40, :], -1.0, Bv[0:40, :],
                    op0=ALU.mult, op1=ALU.subtract)
                nc.vector.scalar_tensor_tensor(
                    fb[BO:BO + 40, :], A[BO:BO + 40, ::-1], -1.0,
                    Bv[BO:BO + 40, ::-1], op0=ALU.mult, op1=ALU.subtract)
                nc.sync.dma_start(c0[40:80, :], fb[BO:BO + 40, :])
                st["c0"] = c0

            def emit_pow(smp):
                # powers of feat, 80-dense; pieces that land at legal partition
                # bases are computed straight into the K-chunk tiles, the rest
                # are placed by SBUF-to-SBUF DMAs.
                st = S[smp]
                c0 = st["c0"]
                c1 = kp.tile([120, NT2], FP16, tag="dc1", bufs=3)
                c2 = kp.tile([120, NT2], FP16, tag="dc2", bufs=3)
                c3 = kp.tile([120, NT2], FP16, tag="dc3", bufs=3)
                c4 = kp.tile([80, NT2], FP16, tag="dc4", bufs=3)
                s2t = kp.tile([80, NT2], FP16, tag="s2t", bufs=2)
                nc.vector.tensor_mul(s2t[:], c0[0:80, :], c0[0:80, :])
                nc.gpsimd.dma_start(c0[80:120, :], s2t[0:40, :])
                nc.gpsimd.dma_start(c1[0:40, :], s2t[40:80, :])
                s3t = kp.tile([80, NT2], FP16, tag="s3t", bufs=2)
                nc.vector.tensor_mul(s3t[:], s2t[:], c0[0:80, :])
                nc.gpsimd.dma_start(c1[40:120, :], s3t[:])
                nc.vector.tensor_mul(c2[0:80, :], s2t[:], s2t[:])      # s4
                s5t = kp.tile([80, NT2], FP16, tag="s5t", bufs=2)
                nc.vector.tensor_mul(s5t[:], s2t[:], s3t[:])
                nc.gpsimd.dma_start(c2[80:120, :], s5t[0:40, :])
                nc.sync.dma_start(c3[0:40, :], s5t[40:80, :])
                s6t = kp.tile([80, NT2], FP16, tag="s6t", bufs=2)
                nc.gpsimd.tensor_mul(s6t[:], s3t[:], s3t[:])
                nc.sync.dma_start(c3[40:120, :], s6t[:])
                nc.vector.tensor_mul(c4[0:80, :], c2[0:80, :], s3t[:])  # s7
                st["dcs"] = [c0, c1, c2, c3, c4]

            def emit_kanmm(smp):
                st = S[smp]
                s0 = smp * T
                dcs = st["dcs"]
                ots = [op_.tile([msz, NT2], FP16, tag=f"ot{mc}", name=f"ot{mc}")
                       for mc, (m0, msz) in enumerate(MCH)]
                for h in range(2):
                    hs = slice(h * NT, (h + 1) * NT)
                    for mc, (m0, msz) in enumerate(MCH):
                        po = psg.tile([128, NT], F32, tag="pp", bufs=2, name="po")
                        for c in range(5):
                            kc = KCHUNK[c]
                            mm(po[0:msz, :], wk[c][:, m0:m0 + msz],
                               dcs[c][0:kc, hs],
                               start=(c == 0), stop=(c == 4))
                        nc.scalar.activation(ots[mc][:, hs], po[0:msz, :],
                                             AF.Sigmoid,
                                             bias=bk[0:msz, mc:mc + 1])
                for mc, (m0, msz) in enumerate(MCH):
                    nc.gpsimd.dma_start(yt_d[m0:m0 + msz, s0:s0 + NT2], ots[mc][:])
                st.clear()

            # ---- software pipeline: iter k runs KANMM(k-3) | L0(k) | L1(k-1)
            # | POW(k-2); KAN matmuls first so the PE queue never head-of-line
            # blocks on the repack DMAs.
            load_x(0)
            for k in range(n_samples + 3):
                if k + 1 < n_samples:
                    load_x(k + 1)
                if 0 <= k - 3:
                    emit_kanmm(k - 3)
                if k < n_samples:
                    emit_l0(k)
                if 0 <= k - 1 < n_samples:
                    emit_l1(k - 1)
                if 0 <= k - 2 < n_samples:
                    emit_pow(k - 2)
    nc.compile()
    return nc


# --------------------------------------------------------------------------
# host entry point
# --------------------------------------------------------------------------
_NC_CACHE = {}


def _get_nc(n_samples=SPB):
    if n_samples not in _NC_CACHE:
        _NC_CACHE[n_samples] = build_nc(n_samples)
    return _NC_CACHE[n_samples]


def make_in_maps(inputs, n_samples=SPB, n_cores=N_CORES):
    x = np.asarray(inputs["x"], dtype=np.float32)
    Wf = fold_weights(inputs)
    in_maps = []
    for c in range(n_cores):
        xc = x[c * n_samples:(c + 1) * n_samples].reshape(n_samples * T, IN_SIZE)
        xt = np.ascontiguousarray(xc.T.astype(np.float16))
        in_maps.append({
            "xt0": np.ascontiguousarray(xt[0:128]),
            "xt1": np.ascontiguousarray(xt[128:256]),
            "x2": np.ascontiguousarray(xt[256:257]),
            **Wf,
        })
    return in_maps


def kernel(**inputs):
    x = np.asarray(inputs["x"], dtype=np.float32)
    assert x.shape == (B, T, IN_SIZE), x.shape
    nc = _get_nc(SPB)
    in_maps = make_in_maps(inputs)
    res = run_bass_kernel_spmd(nc, in_maps, list(range(N_CORES)))
    out = np.empty((B, T, OUT_SIZE), dtype=np.float32)
    for c in range(N_CORES):
        yt = res.results[c]["yt"]  # (257, 8000) f16
        out[c * SPB:(c + 1) * SPB] = (
            yt.T.astype(np.float32) * np.float32(1.2)
        ).reshape(SPB, T, OUT_SIZE)
    return out


if __name__ == "__main__":
    rng = np.random.default_rng(0)
    demo = {
        "x": rng.standard_normal((B, T, IN_SIZE), dtype=np.float32),
        "Wih_f": rng.standard_normal((2, 120, 257), dtype=np.float32) * 0.1,
        "Whh_f": rng.standard_normal((2, 120, 40), dtype=np.float32) * 0.1,
        "bih_f": rng.standard_normal((2, 120), dtype=np.float32) * 0.1,
        "bhh_f": rng.standard_normal((2, 120), dtype=np.float32) * 0.1,
        "Wih_b": rng.standard_normal((2, 120, 257), dtype=np.float32) * 0.1,
        "Whh_b": rng.standard_normal((2, 120, 40), dtype=np.float32) * 0.1,
        "bih_b": rng.standard_normal((2, 120), dtype=np.float32) * 0.1,
        "bhh_b": rng.standard_normal((2, 120), dtype=np.float32) * 0.1,
        "base_weight": rng.standard_normal((257, 80), dtype=np.float32) * 0.1,
        "spline_weight": rng.standard_normal((257, 80, 8), dtype=np.float32) * 0.1,
        "spline_scaler": np.ones((257, 80), dtype=np.float32),
        "slope": np.ones((257,), dtype=np.float32),
        "lengths": np.full((64,), 1000, dtype=np.int32),
    }
    out = kernel(**demo)
    print("kernel ran, out:", out.shape, out.dtype, float(out.min()), float(out.max()))


# revision 9
# speedup vs baseline: 1.0824x; 1.0824x over previous
"""Trainium2 Bass kernel for nn_EnhancementGenerator.

Math: the reference is a (buggy, non-recurrent) bidirectional 2-layer GRU
applied pointwise over (B,T), followed by an efficient-kan KANLinear and
1.2*sigmoid(slope*out).  Everything is row-pointwise except that the
backward direction pairs output row (b,t) with input row (b,T-1-t).

Reformulation:
  * GRU: no recurrence => 4 independent "cells".  Both directions are packed
    into [f(40); b(40)] = partitions 0:40 / 64:104 (partition-base rule);
    the time reversal is applied once at feat-assembly with a reversed
    free-dim access pattern.  h1 is carried negated (h1n = (z1-1)*n1); the
    L1 recurrent weights are negated on host.  x feature 256 (257 = 2*128+1)
    rides as partition 104 of the L1 h-chunk (weight row = Wih_l1[:,256]);
    for L0 and the L1 n-gate it is a K=1 matmul pass.
  * KAN: base-silu + cubic-spline basis == span{silu, 1..f^3, 4 trunc
    cubes}.  Approximated by a degree-7 polynomial in feat (silu fit is
    ~exact; trunc-cube fits give ~8.5e-3 rel overall, budget is 2e-2).
    spl = sum_d P_d f^d: 7 rhs vectors [f..f^7] x 80 feats = 560 K rows,
    DMA-repacked dense into 5 chunks [120,120,120,120,80] so the KAN is
    5 K-chunks x 3 M-blocks of matmul.  P_d / bias folded on host.
  * The final 1.2x scale is applied on host during the f16->f32 upcast.

Layout: features/gates in SBUF partitions, rows in the free dim.  Each core
gets 8 batch samples = 8000 rows; matmuls run at N=500 (PSUM f32 bank limit)
as 2 halves per sample; elementwise ops run full-width N=1000 in fp16.
"""
import os
import sys

for _p in (
    "/root/.axon_site",
    "/root/.axon_site/_ro/trn_rl_repo",
    "/root/.axon_site/_ro/pypackages",
    "/opt/trn_rl_repo",
    "/opt/pypackages",
):
    if os.path.isdir(_p) and _p not in sys.path:
        sys.path.append(_p)

import numpy as np

import concourse.bass as bass
import concourse.tile as tile
from concourse import bacc, mybir
from concourse.bass_utils import run_bass_kernel_spmd

F32 = mybir.dt.float32
FP16 = mybir.dt.float16
AF = mybir.ActivationFunctionType
ALU = mybir.AluOpType

N_CORES = 8
B, T, IN_SIZE, HID, OUT_SIZE = 64, 1000, 257, 40, 257
NT = 500            # psum half width (f32 bank limit 512)
SPB = B // N_CORES  # samples per core
PG = 104            # packed direction block: f at 0:40, b at 64:104
BO = 64             # b-direction partition offset
NPOW = 7            # polynomial degree (rhs = f..f^7)
KCHUNK = [120, 120, 120, 120, 80]   # dense KAN K-chunks (sum = 7*80)
MCH = [(0, 128), (128, 128), (256, 1)]  # KAN output M-blocks

# Degree-7 LSQ fits over the empirical feat distribution (|f|<1):
# silu(f) and the 4 two-sided truncated cubes of the uniform-knot spline.
C_SILU = [6.308492028423155e-07, 0.5000000490243806, 0.24997364172659992,
          -5.742981447318585e-07, -0.02066946270890013, 1.5669095660303758e-06,
          0.0017603356959882746, -1.1728766357584825e-06]
C_TRUNC = {
    (-0.6, "L"): [0.0001651082331822392, 0.000167641210570048,
                  -0.007694731928173372, 0.00250875223865735,
                  0.04856470985596804, -0.032699784691622734,
                  -0.07363537344626084, 0.0637407065220845],
    (-0.2, "L"): [-0.0003715091342959778, -0.005336319702092779,
                  0.03194056417245047, 0.05081939992485063,
                  -0.3584596956621296, 0.36429432612922186,
                  0.07159215401107981, -0.15726428315914862],
    (0.2, "R"): [0.00036529457231786937, -0.005363521000470467,
                 -0.03183079083350338, 0.05104781465712958,
                 0.3580816694009009, 0.3637871708587963,
                 -0.07126037134005792, -0.15693749830364792],
    (0.6, "R"): [-0.00016336387734099217, 0.00017070486860762662,
                 0.007648889140930259, 0.0024753300438013297,
                 -0.048404573931655016, -0.03263337742091405,
                 0.07349986964835029, 0.0637109540665507],
}


# --------------------------------------------------------------------------
# host-side weight folding
# --------------------------------------------------------------------------
def fold_weights(inp):
    from math import comb
    W = {k: np.asarray(v, dtype=np.float64) for k, v in inp.items()}
    out = {}

    # ---- GRU input weights: 6 gate blocks of 104 cols: (l,g) l-major
    wgi = np.zeros((IN_SIZE, 6 * PG))
    for l in range(2):
        for g in range(3):
            c0 = (l * 3 + g) * PG
            wgi[:, c0:c0 + 40] = W["Wih_f"][l][g * 40:(g + 1) * 40].T
            wgi[:, c0 + BO:c0 + BO + 40] = W["Wih_b"][l][g * 40:(g + 1) * 40].T
    out["wgi0"] = wgi[0:128]
    out["wgi1"] = wgi[128:256]
    out["w256"] = wgi[256:257]

    # ---- L1 recurrent weights (negated, blockdiag) + x256 row:
    # blocks r2 | z2 | p3(n)
    wgh = np.zeros((105, 3 * PG))
    for gi_, g in enumerate([0, 1, 2]):
        c0 = gi_ * PG
        wgh[0:40, c0:c0 + 40] = -W["Whh_f"][1][g * 40:(g + 1) * 40].T
        wgh[BO:BO + 40, c0 + BO:c0 + BO + 40] = -W["Whh_b"][1][g * 40:(g + 1) * 40].T
    # x256 contribution to L1 r,z gates rides the h-chunk (row 104)
    for gi_, g in enumerate([0, 1]):
        c0 = gi_ * PG
        wgh[104, c0:c0 + 40] = W["Wih_f"][1][g * 40:(g + 1) * 40, 256]
        wgh[104, c0 + BO:c0 + BO + 40] = W["Wih_b"][1][g * 40:(g + 1) * 40, 256]
    out["wgh"] = wgh

    # ---- GRU biases [105, 8]: cols l*4 + (r, z, bhh_n, bih_n)
    bg = np.zeros((105, 8))
    for l in range(2):
        for gi_ in range(2):
            bg[0:40, l * 4 + gi_] = (W["bih_f"][l][gi_ * 40:(gi_ + 1) * 40]
                                     + W["bhh_f"][l][gi_ * 40:(gi_ + 1) * 40])
            bg[BO:BO + 40, l * 4 + gi_] = (W["bih_b"][l][gi_ * 40:(gi_ + 1) * 40]
                                           + W["bhh_b"][l][gi_ * 40:(gi_ + 1) * 40])
        bg[0:40, l * 4 + 2] = W["bhh_f"][l][80:120]
        bg[BO:BO + 40, l * 4 + 2] = W["bhh_b"][l][80:120]
        bg[0:40, l * 4 + 3] = W["bih_f"][l][80:120]
        bg[BO:BO + 40, l * 4 + 3] = W["bih_b"][l][80:120]
    out["bgru"] = bg

    # ---- KAN: exact truncated-power decomposition, then poly-7 coefficients
    h = 0.4
    t = -2.2 + h * np.arange(12)
    w = W["spline_weight"] * W["spline_scaler"][..., None]          # (257,80,8)
    s = np.zeros((8, 12))
    for m in range(8):
        for k in range(5):
            s[m, m + k] = ((-1) ** k) * comb(4, k) / (6 * h ** 3)
    V = np.einsum("oim,mj->oij", w, s)                              # (257,80,12)
    A = np.zeros((4, 257, 80))
    for j in range(6):
        for d in range(4):
            A[d] += V[:, :, j] * comb(3, d) * ((-t[j]) ** (3 - d))
    tr_mats = {(-0.6, "L"): -V[:, :, 4], (-0.2, "L"): -V[:, :, 5],
               (0.2, "R"): V[:, :, 6], (0.6, "R"): V[:, :, 7]}
    P = np.zeros((NPOW + 1, 257, 80))
    for d in range(4):
        P[d] += A[d]
    bw = W["base_weight"]
    for d in range(NPOW + 1):
        P[d] += bw * C_SILU[d]
        for key, M in tr_mats.items():
            P[d] += M * C_TRUNC[key][d]
    slope = W["slope"]

    # dense chunk weights: flat k = (d-1)*80 + j (j = feat index 0:80)
    Pm = P * slope[None, :, None]                                   # fold slope
    flat = np.concatenate([Pm[d].T for d in range(1, NPOW + 1)], axis=0)  # (560,257)
    o0 = 0
    for c, rows in enumerate(KCHUNK):
        out[f"wk{c}"] = flat[o0:o0 + rows]
        o0 += rows

    bk = np.zeros((128, 3))
    a0 = Pm[0].sum(axis=1)                                          # (257,)
    bk[0:128, 0] = a0[0:128]
    bk[0:128, 1] = a0[128:256]
    bk[0:1, 2] = a0[256:257]
    out["bkan"] = bk

    res = {}
    for k, v in out.items():
        dt = np.float32 if k in ("bgru", "bkan") else np.float16
        res[k] = np.ascontiguousarray(v, dtype=dt)
    return res


# --------------------------------------------------------------------------
# device kernel
# --------------------------------------------------------------------------
def build_nc(n_samples=SPB):
    rows = n_samples * T
    NT2 = 2 * NT
    nc = bacc.Bacc("TRN2", target_bir_lowering=False, debug=False)

    def mm(out, lhsT, rhs, **kw):
        nc.tensor.matmul(out, lhsT, rhs, **kw)

    xt0_d = nc.dram_tensor("xt0", [128, rows], FP16, kind="ExternalInput")
    xt1_d = nc.dram_tensor("xt1", [128, rows], FP16, kind="ExternalInput")
    x2_d = nc.dram_tensor("x2", [1, rows], FP16, kind="ExternalInput")
    wgi0_d = nc.dram_tensor("wgi0", [128, 6 * PG], FP16, kind="ExternalInput")
    wgi1_d = nc.dram_tensor("wgi1", [128, 6 * PG], FP16, kind="ExternalInput")
    w256_d = nc.dram_tensor("w256", [1, 6 * PG], FP16, kind="ExternalInput")
    wgh_d = nc.dram_tensor("wgh", [105, 3 * PG], FP16, kind="ExternalInput")
    wk_d = [nc.dram_tensor(f"wk{c}", [KCHUNK[c], 257], FP16, kind="ExternalInput")
            for c in range(5)]
    bgru_d = nc.dram_tensor("bgru", [105, 8], F32, kind="ExternalInput")
    bkan_d = nc.dram_tensor("bkan", [128, 3], F32, kind="ExternalInput")
    yt_d = nc.dram_tensor("yt", [OUT_SIZE, rows], FP16, kind="ExternalOutput")

    with tile.TileContext(nc) as tc:
        with (
            tc.tile_pool(name="wts", bufs=1) as wp,
            tc.tile_pool(name="xin", bufs=3) as xp,
            tc.tile_pool(name="work", bufs=1) as kp,
            tc.tile_pool(name="outp", bufs=2) as op_,
            tc.tile_pool(name="psg", bufs=1, space="PSUM") as psg,
        ):
            # ---- resident weights
            wgi0 = wp.tile([128, 6 * PG], FP16, tag="wgi0")
            nc.sync.dma_start(wgi0[:], wgi0_d[:])
            wgi1 = wp.tile([128, 6 * PG], FP16, tag="wgi1")
            nc.sync.dma_start(wgi1[:], wgi1_d[:])
            w256 = wp.tile([1, 6 * PG], FP16, tag="w256")
            nc.sync.dma_start(w256[:], w256_d[:])
            wgh = wp.tile([105, 3 * PG], FP16, tag="wgh")
            nc.sync.dma_start(wgh[:], wgh_d[:])
            wk = []
            for c in range(5):
                wt = wp.tile([KCHUNK[c], 257], FP16, tag=f"wk{c}")
                nc.sync.dma_start(wt[:], wk_d[c][:])
                wk.append(wt)
            bg = wp.tile([105, 8], F32, tag="bgru")
            nc.sync.dma_start(bg[:], bgru_d[:])
            bk = wp.tile([128, 3], F32, tag="bkan")
            nc.sync.dma_start(bk[:], bkan_d[:])

            S = [dict() for _ in range(n_samples)]

            def load_x(smp):
                st = S[smp]
                s0 = smp * T
                st["x0"] = xp.tile([128, NT2], FP16, tag="x0", name="x0")
                nc.sync.dma_start(st["x0"][:], xt0_d[:, s0:s0 + NT2])
                st["x1"] = xp.tile([128, NT2], FP16, tag="x1", name="x1")
                nc.sync.dma_start(st["x1"][:], xt1_d[:, s0:s0 + NT2])
                st["x2"] = xp.tile([1, NT2], FP16, tag="x2", name="x2")
                nc.sync.dma_start(st["x2"][:], x2_d[:, s0:s0 + NT2])
                # h-chunk rhs for L1: rows 0:104 = h1n (written in L0),
                # row 104 = x feature 256
                st["htl"] = xp.tile([105, NT2], FP16, tag="htl", name="htl")
                nc.sync.dma_start(st["htl"][104:105, :], x2_d[:, s0:s0 + NT2])

            def gi3(p, st, blk, hs, last_extra=False):
                c0 = blk * PG
                mm(p[:], wgi0[:, c0:c0 + PG], st["x0"][:, hs], start=True, stop=False)
                mm(p[:], wgi1[:, c0:c0 + PG], st["x1"][:, hs], start=False,
                   stop=False)
                if not last_extra:
                    mm(p[:], w256[:, c0:c0 + PG], st["x2"][:, hs], start=False,
                       stop=True)

            def emit_l0(smp):
                st = S[smp]
                rt = kp.tile([PG, NT2], FP16, tag="rt", bufs=2)
                zt = kp.tile([PG, NT2], FP16, tag="zt", bufs=2)
                ut = kp.tile([PG, NT2], FP16, tag="ut", bufs=2)
                for h in range(2):
                    hs = slice(h * NT, (h + 1) * NT)

                    def g_r():
                        ps_r = psg.tile([PG, NT], F32, tag="l0r", name="ps_r")
                        gi3(ps_r, st, 0, hs)
                        nc.scalar.activation(rt[:, hs], ps_r[:], AF.Sigmoid,
                                             bias=bg[0:PG, 0:1])

                    def g_z():
                        ps_z = psg.tile([PG, NT], F32, tag="l0z", name="ps_z")
                        gi3(ps_z, st, 1, hs)
                        nc.scalar.activation(zt[:, hs], ps_z[:], AF.Sigmoid,
                                             bias=bg[0:PG, 1:2])

                    def g_n():
                        ps_n = psg.tile([PG, NT], F32, tag="l0n", name="ps_n")
                        gi3(ps_n, st, 2, hs)
                        nc.vector.scalar_tensor_tensor(
                            ut[:, hs], rt[:, hs], bg[0:PG, 2:3], ps_n[:],
                            op0=ALU.mult, op1=ALU.add)

                    # h1 starts with z: its h0 drain is queued earlier than
                    # n's DVE read, giving the bank-reuse more slack
                    for g in ([g_r, g_z, g_n] if h == 0 else [g_z, g_r, g_n]):
                        g()
                n1 = kp.tile([PG, NT2], FP16, tag="n1", bufs=2)
                nc.scalar.activation(n1[:], ut[:], AF.Tanh, bias=bg[0:PG, 3:4])
                # h1n = (z-1)*n1 = -h1, written into the L1 h-chunk rhs
                nc.vector.scalar_tensor_tensor(
                    st["htl"][0:PG, :], zt[:], 1.0, n1[:],
                    op0=ALU.subtract, op1=ALU.mult)

            def emit_l1(smp):
                st = S[smp]
                r2t = kp.tile([PG, NT2], FP16, tag="r2t", bufs=2)
                z2t = kp.tile([PG, NT2], FP16, tag="z2t", bufs=2)
                t2t = kp.tile([PG, NT2], FP16, tag="t2t", bufs=2)
                vt = kp.tile([PG, NT2], FP16, tag="vt", bufs=2)
                for h in range(2):
                    hs = slice(h * NT, (h + 1) * NT)
                    ps_r2 = psg.tile([PG, NT], F32, tag="l1r", name="ps_r2")
                    gi3(ps_r2, st, 3, hs, last_extra=True)
                    mm(ps_r2[:], wgh[:, 0:PG], st["htl"][:, hs], start=False,
                       stop=True)
                    ps_z2 = psg.tile([PG, NT], F32, tag="l1z", name="ps_z2")
                    gi3(ps_z2, st, 4, hs, last_extra=True)
                    mm(ps_z2[:], wgh[:, PG:2 * PG], st["htl"][:, hs], start=False,
                       stop=True)
                    ps_n2 = psg.tile([PG, NT], F32, tag="l1n", name="ps_n2")
                    gi3(ps_n2, st, 5, hs)
                    ps_p3 = psg.tile([128, NT], F32, tag="pp", bufs=2, name="ps_p3")
                    mm(ps_p3[0:PG, :], wgh[0:104, 2 * PG:3 * PG],
                       st["htl"][0:104, hs], start=True, stop=True)
                    nc.scalar.activation(r2t[:, hs], ps_r2[:], AF.Sigmoid,
                                         bias=bg[0:PG, 4:5])
                    nc.scalar.activation(z2t[:, hs], ps_z2[:], AF.Sigmoid,
                                         bias=bg[0:PG, 5:6])
                    nc.vector.scalar_tensor_tensor(
                        t2t[:, hs], ps_p3[0:PG, :], bg[0:PG, 6:7], r2t[:, hs],
                        op0=ALU.add, op1=ALU.mult)
                    nc.vector.tensor_add(vt[:, hs], t2t[:, hs], ps_n2[:])
                n2 = kp.tile([PG, NT2], FP16, tag="n2", bufs=2)
                nc.scalar.activation(n2[:], vt[:], AF.Tanh, bias=bg[0:PG, 7:8])
                # hf = (1-z2)*n2 + z2*h1 = -(A + Bv), A=(z2-1)*n2, Bv=z2*h1n
                A = kp.tile([PG, NT2], FP16, tag="A", bufs=2)
                nc.vector.scalar_tensor_tensor(
                    A[:], z2t[:], 1.0, n2[:], op0=ALU.subtract, op1=ALU.mult)
                Bv = kp.tile([PG, NT2], FP16, tag="Bv", bufs=2)
                nc.vector.tensor_mul(Bv[:], z2t[:], st["htl"][0:PG, :])
                # feat goes dense [80] into KAN chunk 0: f-part written directly,
                # b-part (time-reversed) via a legal base-64 scratch + 1 DMA.
                c0 = kp.tile([120, NT2], FP16, tag="dc0", bufs=3)
                fb = kp.tile([PG, NT2], FP16, tag="fb", bufs=2)
                nc.vector.scalar_tensor_tensor(
                    c0[0:40, :], A[0:40, :], -1.0, Bv[0:40, :],
                    op0=ALU.mult, op1=ALU.subtract)
                nc.vector.scalar_tensor_tensor(
                    fb[BO:BO + 40, :], A[BO:BO + 40, ::-1], -1.0,
                    Bv[BO:BO + 40, ::-1], op0=ALU.mult, op1=ALU.subtract)
                nc.sync.dma_start(c0[40:80, :], fb[BO:BO + 40, :])
                st["c0"] = c0

            def emit_pow(smp):
                # powers of feat, 80-dense; pieces that land at legal partition
                # bases are computed straight into the K-chunk tiles, the rest
                # are placed by SBUF-to-SBUF DMAs.
                st = S[smp]
                c0 = st["c0"]
                c1 = kp.tile([120, NT2], FP16, tag="dc1", bufs=3)
                c2 = kp.tile([120, NT2], FP16, tag="dc2", bufs=3)
                c3 = kp.tile([120, NT2], FP16, tag="dc3", bufs=3)
                c4 = kp.tile([80, NT2], FP16, tag="dc4", bufs=3)
                s2t = kp.tile([80, NT2], FP16, tag="s2t", bufs=2)
                nc.vector.tensor_mul(s2t[:], c0[0:80, :], c0[0:80, :])
                nc.gpsimd.dma_start(c0[80:120, :], s2t[0:40, :])
                nc.gpsimd.dma_start(c1[0:40, :], s2t[40:80, :])
                s3t = kp.tile([80, NT2], FP16, tag="s3t", bufs=2)
                nc.vector.tensor_mul(s3t[:], s2t[:], c0[0:80, :])
                nc.gpsimd.dma_start(c1[40:120, :], s3t[:])
                nc.vector.tensor_mul(c2[0:80, :], s2t[:], s2t[:])      # s4
                s5t = kp.tile([80, NT2], FP16, tag="s5t", bufs=2)
                nc.vector.tensor_mul(s5t[:], s2t[:], s3t[:])
                nc.gpsimd.dma_start(c2[80:120, :], s5t[0:40, :])
                nc.sync.dma_start(c3[0:40, :], s5t[40:80, :])
                s6t = kp.tile([80, NT2], FP16, tag="s6t", bufs=2)
                nc.gpsimd.tensor_mul(s6t[:], s3t[:], s3t[:])
                nc.sync.dma_start(c3[40:120, :], s6t[:])
                nc.vector.tensor_mul(c4[0:80, :], c2[0:80, :], s3t[:])  # s7
                st["dcs"] = [c0, c1, c2, c3, c4]

            def emit_kanmm(smp):
                st = S[smp]
                s0 = smp * T
                dcs = st["dcs"]
                ots = [op_.tile([msz, NT2], FP16, tag=f"ot{mc}", name=f"ot{mc}")
                       for mc, (m0, msz) in enumerate(MCH)]
                for h in range(2):
                    hs = slice(h * NT, (h + 1) * NT)
                    for mc, (m0, msz) in enumerate(MCH):
                        po = psg.tile([128, NT], F32, tag="pp", bufs=2, name="po")
                        for c in range(5):
                            kc = KCHUNK[c]
                            mm(po[0:msz, :], wk[c][:, m0:m0 + msz],
                               dcs[c][0:kc, hs],
                               start=(c == 0), stop=(c == 4))
                        nc.scalar.activation(ots[mc][:, hs], po[0:msz, :],
                                             AF.Sigmoid,
                                             bias=bk[0:msz, mc:mc + 1])
                for mc, (m0, msz) in enumerate(MCH):
                    nc.gpsimd.dma_start(yt_d[m0:m0 + msz, s0:s0 + NT2], ots[mc][:])
                st.clear()

            # ---- software pipeline: iter k runs KANMM(k-3) | L0(k) | L1(k-1)
            # | POW(k-2); KAN matmuls first so the PE queue never head-of-line
            # blocks on the repack DMAs.
            load_x(0)
            # HAM warmup: back-to-back matmuls on sample 0's freshly loaded
            # x tile flip the PE clock gate to 8/8 (2.4 GHz); keying them on
            # x0 means the first real matmul follows with zero gap.
            warm = psg.tile([128, NT], F32, tag="pp", bufs=2, name="warm")
            for _ in range(24):
                mm(warm[:, :], wgi0[:, 0:128], S[0]["x0"][:, 0:NT],
                   start=True, stop=True)
            for k in range(n_samples + 3):
                if k + 1 < n_samples:
                    load_x(k + 1)
                if 0 <= k - 3:
                    emit_kanmm(k - 3)
                if k < n_samples:
                    emit_l0(k)
                if 0 <= k - 1 < n_samples:
                    emit_l1(k - 1)
                if 0 <= k - 2 < n_samples:
                    emit_pow(k - 2)
    nc.compile()
    return nc


# --------------------------------------------------------------------------
# host entry point
# --------------------------------------------------------------------------
_NC_CACHE = {}


def _get_nc(n_samples=SPB):
    if n_samples not in _NC_CACHE:
        _NC_CACHE[n_samples] = build_nc(n_samples)
    return _NC_CACHE[n_samples]


def make_in_maps(inputs, n_samples=SPB, n_cores=N_CORES):
    x = np.asarray(inputs["x"], dtype=np.float32)
    Wf = fold_weights(inputs)
    in_maps = []
    for c in range(n_cores):
        xc = x[c * n_samples:(c + 1) * n_samples].reshape(n_samples * T, IN_SIZE)
        xt = np.ascontiguousarray(xc.T.astype(np.float16))
        in_maps.append({
            "xt0": np.ascontiguousarray(xt[0:128]),
            "xt1": np.ascontiguousarray(xt[128:256]),
            "x2": np.ascontiguousarray(xt[256:257]),
            **Wf,
        })
    return in_maps


def kernel(**inputs):
    x = np.asarray(inputs["x"], dtype=np.float32)
    assert x.shape == (B, T, IN_SIZE), x.shape
    nc = _get_nc(SPB)
    in_maps = make_in_maps(inputs)
    res = run_bass_kernel_spmd(nc, in_maps, list(range(N_CORES)))
    out = np.empty((B, T, OUT_SIZE), dtype=np.float32)
    for c in range(N_CORES):
        yt = res.results[c]["yt"]  # (257, 8000) f16
        out[c * SPB:(c + 1) * SPB] = (
            yt.T.astype(np.float32) * np.float32(1.2)
        ).reshape(SPB, T, OUT_SIZE)
    return out


if __name__ == "__main__":
    rng = np.random.default_rng(0)
    demo = {
        "x": rng.standard_normal((B, T, IN_SIZE), dtype=np.float32),
        "Wih_f": rng.standard_normal((2, 120, 257), dtype=np.float32) * 0.1,
        "Whh_f": rng.standard_normal((2, 120, 40), dtype=np.float32) * 0.1,
        "bih_f": rng.standard_normal((2, 120), dtype=np.float32) * 0.1,
        "bhh_f": rng.standard_normal((2, 120), dtype=np.float32) * 0.1,
        "Wih_b": rng.standard_normal((2, 120, 257), dtype=np.float32) * 0.1,
        "Whh_b": rng.standard_normal((2, 120, 40), dtype=np.float32) * 0.1,
        "bih_b": rng.standard_normal((2, 120), dtype=np.float32) * 0.1,
        "bhh_b": rng.standard_normal((2, 120), dtype=np.float32) * 0.1,
        "base_weight": rng.standard_normal((257, 80), dtype=np.float32) * 0.1,
        "spline_weight": rng.standard_normal((257, 80, 8), dtype=np.float32) * 0.1,
        "spline_scaler": np.ones((257, 80), dtype=np.float32),
        "slope": np.ones((257,), dtype=np.float32),
        "lengths": np.full((64,), 1000, dtype=np.int32),
    }
    out = kernel(**demo)
    print("kernel ran, out:", out.shape, out.dtype, float(out.min()), float(out.max()))
